# revision 5
# baseline (speedup 1.0000x reference)
"""Single-head causal attention (B=8, T=4096, C=384, H=64) on 8 trn2 cores.

Sharding: data-parallel over batch — one example per NeuronCore.

Per-core dataflow (all matmul inputs bf16, fp32 PSUM accumulation):
  - host pre-transposes x to xT [C, T] bf16; Wq is pre-scaled by
    C**-0.5 * log2(e) so scores come out in the log2 domain.
  - projections compute qT/kT [64, T] (packed [Wq|Wk] -> one M=128 matmul)
    and vT [64, T]; vT gets a ones-row appended and is PE-transposed to
    v_aug [T-blocks, 128, 65] (ones column -> softmax denominator rides
    the PV matmul for free).
  - main loop over 8 query superblocks (512 wide) x causal key blocks
    (128 wide): S^T = kT_blk^T @ qT in PSUM, ScalarE exp2 straight out of
    PSUM into bf16 P^T (no max-subtraction: |scores| <= ~5), causal mask
    via affine_select on diagonal blocks, then O^T += v_aug^T @ P^T.
  - finalize: PE-transpose O^T back to natural layout, divide by the
    denominator column, DMA out.
"""

import math

import ml_dtypes
import numpy as np

B, T, C, H = 8, 4096, 384, 64
P = 128
TB = T // P            # 32 key blocks
SB = T // 512          # 8 query superblocks
CO = C // P            # 3 contraction chunks

_CACHE = {}


def _build():
    import concourse.bass as bass
    import concourse.mybir as mybir
    import concourse.tile as tile
    from concourse import bacc
    from concourse.bass import ts
    from concourse.masks import make_identity

    fp32 = mybir.dt.float32
    bf16 = mybir.dt.bfloat16
    LN2 = float(np.log(2.0))

    nc = bacc.Bacc(name="head_attn")
    xT_d = nc.dram_tensor("xt", [C, T], bf16, kind="ExternalInput")
    wqk_d = nc.dram_tensor("wqk", [C, 2 * H], bf16, kind="ExternalInput")
    wv_d = nc.dram_tensor("wv", [C, H], bf16, kind="ExternalInput")
    out_d = nc.dram_tensor("out", [T, H], fp32, kind="ExternalOutput")

    with tile.TileContext(nc) as tc:
        with (
            tc.tile_pool(name="const", bufs=1) as cpool,
            tc.tile_pool(name="big", bufs=1) as big,
            tc.tile_pool(name="pt", bufs=3) as ptp,
            tc.tile_pool(name="ps", bufs=3, space="PSUM") as psp,
            tc.tile_pool(name="po", bufs=2, space="PSUM") as pop,
            tc.tile_pool(name="ptr", bufs=2, space="PSUM") as ptrp,
        ):
            ident_b = cpool.tile([P, P], bf16, tag="idb")
            make_identity(nc, ident_b[:])
            ident_f = cpool.tile([P, P], fp32, tag="idf")
            make_identity(nc, ident_f[:])

            wqk = cpool.tile([P, CO, 2 * H], bf16, tag="wqk")
            nc.sync.dma_start(wqk[:], wqk_d.rearrange("(o p) m -> p o m", p=P))
            wv = cpool.tile([P, CO, H], bf16, tag="wv")
            nc.sync.dma_start(wv[:], wv_d.rearrange("(o p) m -> p o m", p=P))

            xT = big.tile([P, CO, T], bf16, tag="xT")
            xT_src = xT_d.rearrange("(o p) t -> p o t", p=P)
            for sb in range(SB):
                nc.sync.dma_start(xT[:, :, ts(sb, 512)], xT_src[:, :, ts(sb, 512)])

            qT = big.tile([H, T], bf16, tag="qT")
            kT = big.tile([H, T], bf16, tag="kT")
            vT = big.tile([H + 1, T], bf16, tag="vT")
            nc.vector.memset(vT[H : H + 1, :], 1.0)

            # projections: [Wq|Wk] packed -> psum rows 0:64 = qT, 64:128 = kT
            for sb in range(SB):
                pqk = psp.tile([P, 512], fp32, tag="ps")
                for ci in range(CO):
                    nc.tensor.matmul(
                        pqk[:],
                        wqk[:, ci, :],
                        xT[:, ci, ts(sb, 512)],
                        start=(ci == 0),
                        stop=(ci == CO - 1),
                    )
                nc.vector.tensor_copy(qT[:, ts(sb, 512)], pqk[0:H, :])
                nc.vector.tensor_copy(kT[:, ts(sb, 512)], pqk[H : 2 * H, :])
            for sb in range(SB):
                pv = psp.tile([P, 512], fp32, tag="ps")
                for ci in range(CO):
                    nc.tensor.matmul(
                        pv[0:H, :],
                        wv[:, ci, :],
                        xT[:, ci, ts(sb, 512)],
                        start=(ci == 0),
                        stop=(ci == CO - 1),
                    )
                nc.vector.tensor_copy(vT[0:H, ts(sb, 512)], pv[0:H, :])

            # v_aug[j] = [v_block | ones] : [128, 65]
            vaug = big.tile([P, TB, H + 1], bf16, tag="vaug")
            for j in range(TB):
                ptrb = ptrp.tile([P, 512], bf16, tag="tr")
                nc.tensor.transpose(
                    ptrb[:, 0 : H + 1], vT[:, ts(j, P)], ident_b[0 : H + 1, 0 : H + 1]
                )
                nc.vector.tensor_copy(vaug[:, j, :], ptrb[:, 0 : H + 1])

            oT = big.tile([H + 1, T], fp32, tag="oT")

            for i in range(SB):
                po = pop.tile([P, 512], fp32, tag="po")
                nj = 4 * i + 4
                for j in range(nj):
                    ps = psp.tile([P, 512], fp32, tag="ps")
                    nc.tensor.matmul(
                        ps[:],
                        kT[:, ts(j, P)],
                        qT[:, ts(i, 512)],
                        start=True,
                        stop=True,
                    )
                    pt = ptp.tile([P, 512], bf16, tag="pt")
                    nc.scalar.activation(
                        pt[:], ps[:], mybir.ActivationFunctionType.Exp, scale=LN2
                    )
                    d = j - 4 * i
                    if d >= 0:
                        # zero where key > query: keep iff col >= row + 128*d
                        nc.gpsimd.affine_select(
                            out=pt[:],
                            in_=pt[:],
                            compare_op=mybir.AluOpType.is_ge,
                            fill=0.0,
                            base=-P * d,
                            pattern=[[1, 512]],
                            channel_multiplier=-1,
                        )
                    nc.tensor.matmul(
                        po[0 : H + 1, :],
                        vaug[:, j, :],
                        pt[:],
                        start=(j == 0),
                        stop=(j == nj - 1),
                    )
                nc.vector.tensor_copy(oT[:, ts(i, 512)], po[0 : H + 1, :])

            # transpose back to [T, 65], normalize, store
            osb = big.tile([P, TB, H], fp32, tag="osb")
            rec = cpool.tile([P, TB], fp32, tag="rec")
            for j in range(TB):
                ptr = ptrp.tile([P, 512], fp32, tag="tr")
                nc.tensor.transpose(
                    ptr[:, 0 : H + 1], oT[:, ts(j, P)], ident_f[0 : H + 1, 0 : H + 1]
                )
                nc.vector.reciprocal(rec[:, j : j + 1], ptr[:, H : H + 1])
                nc.vector.tensor_scalar_mul(
                    osb[:, j, :], ptr[:, 0:H], rec[:, j : j + 1]
                )
            nc.sync.dma_start(out_d.rearrange("(j p) h -> p j h", p=P), osb[:])

    nc.compile()
    return nc


def kernel(x, Wk, Wq, Wv):
    from concourse.bass_utils import run_bass_kernel_spmd

    if "nc" not in _CACHE:
        _CACHE["nc"] = _build()
    nc = _CACHE["nc"]

    bf = ml_dtypes.bfloat16
    scale = (C ** -0.5) * (1.0 / math.log(2.0))  # fold softmax scale + log2(e)
    wqk = np.concatenate(
        [np.asarray(Wq, np.float32) * scale, np.asarray(Wk, np.float32)], axis=1
    ).astype(bf)
    wv = np.asarray(Wv, np.float32).astype(bf)
    x = np.asarray(x, np.float32)

    in_maps = [
        {
            "xt": np.ascontiguousarray(x[b].T).astype(bf),
            "wqk": wqk,
            "wv": wv,
        }
        for b in range(B)
    ]
    res = run_bass_kernel_spmd(nc, in_maps, core_ids=list(range(B)))
    return np.stack([r["out"] for r in res.results]).astype(np.float32)
